# revision 19
# baseline (speedup 1.0000x reference)
"""MiniBatch K-means (1 iteration) on 8 Trainium2 NeuronCores.

Strategy (data-parallel over points, per sharding hint):
  - Shard X along N across 8 cores (62500 points each, zero-padded to
    62976 = 492 tiles of 128 points).
  - Per 128-point tile on each core:
      mm1:  q[n,k] = BETA*(c2[k]/2 - x_n.c_k)  (argmin_k == argmin dist),
            computed as two full-rate bf16 matmuls accumulating in PSUM
            (hi/lo error compensation, see below)
      DVE:  m[n] = min_k q[n,k]
      ACT:  onehot[n,k] = exp(m - q)        (=1 at argmin, ~0 elsewhere)
      mm2:  S[65,512] += [X|1]_tile.T @ onehot   (float32r, PSUM accumulation
            across all tiles; rows 0..63 = sums^T, row 64 = counts)
  - Host: sum the 8 per-core S partials, divide, transpose.

Precision: bf16 matmuls stream at full PE rate but rounding X/C to bf16
perturbs distances by ~4e-3 relative, flipping too many near-boundary
assignments. fp32 matmuls are exact but stream at 1/4 rate. Instead we
split x = xh + xl and c = ch + cl (bf16 hi/lo) and compute
    x.c ~= (xh+xl).ch + xh.cl        (only xl.cl ~ 2^-18 is dropped)
  mm1a: lhsT = [xh^T; xl^T] (128 rows), rhs = [ch; ch]        (start)
  mm1b: lhsT = [xh^T; 1;1;1] (67 rows), rhs = [cl; c2 hi/mid/lo] (stop)
The BETA*c2/2 row is split into three bf16 terms so its absolute error
stays ~2^-24 of its ~2e6 magnitude. Net distance error ~1e-4 in h units,
i.e. fp32-reference-level assignment fidelity at bf16 speed.

mm2 runs in float32r (fp32 bit layout, TF32-like precision, full rate):
one_hot values are exact 0/1 and sums average out the 1.6e-4 rounding.

Both X layouts are packed host-side into per-slab tensors (12 tiles per
DMA) because each dma_start costs ~1.2us of sequencer/HWDGE issue time.
Padded points have an all-zero [X|1] row in mm2's operand, so they
contribute nothing to S regardless of their (garbage) onehot row.
"""

import numpy as np

N, D, K = 500000, 64, 512
NCORES = 8
NS = N // NCORES            # 62500 points per core
PT = 128                    # points per tile (partition dim)
TPS = 12                    # tiles per DMA slab
TPG = 1                     # tiles per fused reduce group
GPS = TPS // TPG            # reduce groups per slab
NSLAB = -(-NS // (PT * TPS))  # 41 slabs
NTP = NSLAB * TPS           # 492 tiles
NPAD = NTP * PT             # 62976 padded points per core
DA = D + 1                  # 65: X augmented with ones column
DH = D + 3                  # 67: xh rows + three ones rows (c2 hi/mid/lo)
XTF = TPS * PT              # 1536 columns of X^T-part per slab
BETA = 65536.0

_CACHE: dict = {}


def _build_nc():
    from contextlib import ExitStack

    import concourse.bacc as bacc
    import concourse.tile as tile
    from concourse import mybir

    f32 = mybir.dt.float32
    f32r = mybir.dt.float32r
    bf16 = mybir.dt.bfloat16

    nc = bacc.Bacc("TRN2", target_bir_lowering=False, debug=False)

    xall = nc.dram_tensor("xall", [PT, NSLAB, XTF], bf16, kind="ExternalInput")
    xht = nc.dram_tensor("xht", [DH, NSLAB, XTF], bf16, kind="ExternalInput")
    xa = nc.dram_tensor("xa", [PT, NSLAB, TPS * DA], f32r, kind="ExternalInput")
    cha = nc.dram_tensor("cha", [PT, K], bf16, kind="ExternalInput")
    clb = nc.dram_tensor("clb", [DH, K], bf16, kind="ExternalInput")
    sout = nc.dram_tensor("sout", [DA, K], f32, kind="ExternalOutput")

    with tile.TileContext(nc) as tc, ExitStack() as ctx:
        const = ctx.enter_context(tc.tile_pool(name="const", bufs=1))
        ld = ctx.enter_context(tc.tile_pool(name="ld", bufs=3))
        ohp = ctx.enter_context(tc.tile_pool(name="oh", bufs=3))
        mred = ctx.enter_context(tc.tile_pool(name="mred", bufs=4))
        gp = ctx.enter_context(tc.tile_pool(name="g", bufs=6, space="PSUM"))
        sp = ctx.enter_context(tc.tile_pool(name="s", bufs=1, space="PSUM"))

        cha_sb = const.tile([PT, K], bf16)
        nc.sync.dma_start(cha_sb[:], cha[:])
        clb_sb = const.tile([DH, K], bf16)
        nc.sync.dma_start(clb_sb[:], clb[:])

        s_ps = sp.tile([DA, K], f32)  # lives across the whole loop

        NG = NSLAB * GPS  # total reduce groups
        slabs = [None] * NSLAB  # (xall_t, xht_t, xa_t) per slab
        # software pipeline: drain group g-PIPE while issuing mm1s for group
        # g, so the act->mm2->mm1->reduce->act critical cycle spreads over
        # PIPE+1 groups
        PIPE = 4
        pending = []  # [(g_ps, slab_idx, gg)] awaiting reduce/act/mm2

        def drain(pend, last):
            g_ps, si, gg = pend
            _, _, xa_t = slabs[si]
            m_t = mred.tile([PT, TPG], f32)
            nc.vector.tensor_reduce(
                out=m_t[:],
                in_=g_ps[:],
                axis=mybir.AxisListType.X,
                op=mybir.AluOpType.min,
            )
            ohs = []
            for t in range(TPG):
                # one tile per t: a shared [PT, TPG, K] tile would create a
                # false tile-granularity WAR between act(t=1)'s write and
                # mm2(t=0)'s read, serializing ACT behind PE
                oh_t = ohp.tile([PT, K], f32r, tag=f"oh{t}")
                nc.scalar.activation(
                    out=oh_t[:],
                    in_=g_ps[:, t, :],
                    func=mybir.ActivationFunctionType.Exp,
                    bias=m_t[:, t : t + 1],
                    scale=-1.0,
                )
                ohs.append(oh_t)
            for t in range(TPG):
                tt = gg * TPG + t
                nc.tensor.matmul(
                    s_ps[:],
                    xa_t[:, tt * DA : (tt + 1) * DA],
                    ohs[t][:],
                    start=(drain.first and t == 0),
                    stop=(last and t == TPG - 1),
                )
            drain.first = False

        drain.first = True

        for g in range(NG):
            si, gg = divmod(g, GPS)
            if gg == 0:
                xall_t = ld.tile([PT, XTF], bf16, tag="xall")
                nc.sync.dma_start(xall_t[:], xall[:, si, :])
                xht_t = ld.tile([DH, XTF], bf16, tag="xht")
                nc.sync.dma_start(xht_t[:], xht[:, si, :])
                xa_t = ld.tile([PT, TPS * DA], f32r, tag="xa")
                nc.sync.dma_start(xa_t[:], xa[:, si, :])
                slabs[si] = (xall_t, xht_t, xa_t)

            xall_t, xht_t, _ = slabs[si]
            g_ps = gp.tile([PT, TPG, K], f32)
            for t in range(TPG):
                tt = gg * TPG + t
                nc.tensor.matmul(
                    g_ps[:, t, :],
                    xall_t[:, tt * PT : (tt + 1) * PT],
                    cha_sb[:],
                    start=True,
                    stop=False,
                )
                nc.tensor.matmul(
                    g_ps[:, t, :],
                    xht_t[:, tt * PT : (tt + 1) * PT],
                    clb_sb[:],
                    start=False,
                    stop=True,
                )

            pending.append((g_ps, si, gg))
            if len(pending) > PIPE:
                drain(pending.pop(0), last=False)

        while pending:
            drain(pending.pop(0), last=(len(pending) == 0))

        s_sb = const.tile([DA, K], f32)
        nc.vector.tensor_copy(s_sb[:], s_ps[:])
        nc.sync.dma_start(sout[:], s_sb[:])

    nc.compile()
    return nc


def _get_nc():
    if "nc" not in _CACHE:
        _CACHE["nc"] = _build_nc()
    return _CACHE["nc"]


def build_in_maps(X, idx):
    import ml_dtypes

    bf = ml_dtypes.bfloat16

    C = X[idx]  # [K, D] float32
    c2 = 0.5 * BETA * np.einsum(
        "kd,kd->k", C.astype(np.float64), C.astype(np.float64)
    )

    cb = (-BETA) * C.T.astype(np.float64)  # [D, K]
    ch = cb.astype(bf)
    cl = (cb - ch.astype(np.float64)).astype(bf)
    c2a = c2.astype(bf)
    c2b = (c2 - c2a.astype(np.float64)).astype(bf)
    c2c = (c2 - c2a.astype(np.float64) - c2b.astype(np.float64)).astype(bf)

    cha_np = np.concatenate([ch, ch], axis=0)  # [128, K]
    clb_np = np.concatenate(
        [cl, c2a[None], c2b[None], c2c[None]], axis=0
    )  # [67, K]

    in_maps = []
    for c in range(NCORES):
        xs = X[c * NS : (c + 1) * NS]  # [NS, D] float32
        xh = xs.astype(bf)
        xl = (xs.astype(np.float64) - xh.astype(np.float64)).astype(bf)

        # [128, NPAD] bf16: rows 0..63 xh^T, rows 64..127 xl^T
        xall_np = np.zeros((PT, NPAD), bf)
        xall_np[:D, :NS] = xh.T
        xall_np[D : 2 * D, :NS] = xl.T
        # [67, NPAD] bf16: rows 0..63 xh^T, rows 64..66 ones
        xht_np = np.zeros((DH, NPAD), bf)
        xht_np[:D, :NS] = xh.T
        xht_np[D:, :NS] = 1.0

        # point-major [X|1], tiled: [128, NTP, DA] f32 (pad rows all-zero)
        xa_np = np.zeros((NPAD, DA), np.float32)
        xa_np[:NS, :D] = xs
        xa_np[:NS, D] = 1.0
        xa_tiled = np.ascontiguousarray(
            xa_np.reshape(NTP, PT, DA).transpose(1, 0, 2)
        ).reshape(PT, NSLAB, TPS * DA)

        in_maps.append(
            {
                "xall": xall_np.reshape(PT, NSLAB, XTF),
                "xht": xht_np.reshape(DH, NSLAB, XTF),
                "xa": xa_tiled,
                "cha": cha_np,
                "clb": clb_np,
            }
        )
    return in_maps


def kernel(X, init_idx):
    from concourse.bass_utils import run_bass_kernel_spmd

    X = np.ascontiguousarray(np.asarray(X, dtype=np.float32))
    idx = np.asarray(init_idx).astype(np.int64)

    in_maps = build_in_maps(X, idx)
    _CACHE["in_maps"] = in_maps

    # Build a fresh Bass module per call: executing via run_bass_kernel_spmd
    # mutates the module, and re-running a previously-executed one crashes
    # the device (NRT_EXEC_UNIT_UNRECOVERABLE).
    nc = _build_nc()
    res = run_bass_kernel_spmd(nc, in_maps, core_ids=list(range(NCORES)))

    S = np.zeros((DA, K), np.float64)
    for r in res.results:
        S += r["sout"].astype(np.float64)

    counts = S[D, :]                      # [K]
    sums = S[:D, :]                       # [D, K]
    out = (sums / np.maximum(counts, 1.0)).T.astype(np.float32)
    return out
